# revision 2
# baseline (speedup 1.0000x reference)
"""AWGN channel kernel for Trainium2: y = x + sqrt(1/SNR) * noise.

Full inputs x, noise: (16384, 4096) float32. Row-sharded across 8
NeuronCores (data parallel, 2048 rows/core); each core streams 16
chunks of [128, 4096] through SBUF and computes the fused
(noise * STD) + x in one DVE scalar_tensor_tensor op per chunk.
"""

import numpy as np

N_CORES = 8
ROWS, COLS = 16384, 4096
SHARD_ROWS = ROWS // N_CORES  # 2048
P = 128
N_CHUNKS = SHARD_ROWS // P  # 16
SNR = 10.0
STD = float(np.sqrt(1.0 / SNR))

_cache = {}


def _build():
    if "nc" in _cache:
        return _cache["nc"]

    import concourse.tile as tile
    from concourse import bacc, mybir

    nc = bacc.Bacc(
        "TRN2",
        target_bir_lowering=False,
        debug=False,
        num_devices=N_CORES,
    )
    x_ap = nc.dram_tensor(
        "x", [SHARD_ROWS, COLS], mybir.dt.float32, kind="ExternalInput"
    ).ap()
    n_ap = nc.dram_tensor(
        "noise", [SHARD_ROWS, COLS], mybir.dt.float32, kind="ExternalInput"
    ).ap()
    y_ap = nc.dram_tensor(
        "y", [SHARD_ROWS, COLS], mybir.dt.float32, kind="ExternalOutput"
    ).ap()

    with tile.TileContext(nc) as tc:
        with (
            tc.tile_pool(name="xp", bufs=3) as xp,
            tc.tile_pool(name="npool", bufs=3) as npool,
            tc.tile_pool(name="yp", bufs=3) as yp,
        ):
            for c in range(N_CHUNKS):
                r0 = c * P
                xt = xp.tile([P, COLS], mybir.dt.float32)
                nt = npool.tile([P, COLS], mybir.dt.float32)
                yt = yp.tile([P, COLS], mybir.dt.float32)
                nc.sync.dma_start(out=xt[:], in_=x_ap[r0 : r0 + P, :])
                nc.sync.dma_start(out=nt[:], in_=n_ap[r0 : r0 + P, :])
                nc.vector.scalar_tensor_tensor(
                    out=yt[:],
                    in0=nt[:],
                    scalar=STD,
                    in1=xt[:],
                    op0=mybir.AluOpType.mult,
                    op1=mybir.AluOpType.add,
                )
                nc.scalar.dma_start(out=y_ap[r0 : r0 + P, :], in_=yt[:])

    nc.compile()
    _cache["nc"] = nc
    return nc


def _run(x, noise, trace=False, tmpdir=None):
    from concourse.bass_utils import run_bass_kernel_spmd

    nc = _build()
    x = np.ascontiguousarray(x, dtype=np.float32)
    noise = np.ascontiguousarray(noise, dtype=np.float32)
    in_maps = [
        {
            "x": x[i * SHARD_ROWS : (i + 1) * SHARD_ROWS],
            "noise": noise[i * SHARD_ROWS : (i + 1) * SHARD_ROWS],
        }
        for i in range(N_CORES)
    ]
    res = run_bass_kernel_spmd(
        nc, in_maps, list(range(N_CORES)), trace=trace, tmpdir=tmpdir
    )
    out = np.concatenate([res.results[i]["y"] for i in range(N_CORES)], axis=0)
    return out, res


def kernel(x, noise):
    out, _ = _run(x, noise)
    return out
